# revision 43
# baseline (speedup 1.0000x reference)
"""Trainium2 Bass kernel for CausalSelfAttention (B=2, T=2048, D=1024, H=16).

Sharding (8 cores): Megatron-style tensor parallel. Core c owns heads
{2c, 2c+1}: column-parallel c_attn (384 of 3072 output features),
full attention for its 2 heads x 2 batches, row-parallel c_proj
(128 of 1024 contraction rows). Host sums the 8 partial outputs and
adds b_proj.

Key optimization vs the dense version: the attention_mask zeroes ~50%
of tokens. Masked keys contribute nothing (key-masked), and masked
queries output exactly b_proj (their rows are zeroed before c_proj).
Since queries and keys share the same valid subset in order, packing
the valid tokens keeps the attention EXACTLY causal in packed space.
The kernel therefore runs on ~1024+pad tokens per batch instead of
2048: QKV/proj work halves, attention/softmax work quarters. The host
packs inputs / scatters outputs; the kernel is compile-specialized on
the per-batch packed tile counts (rebuilt if the mask pattern changes).

Device algorithm (per core), matmuls bf16, softmax f32:
  1. qkv^T = Wslice^T @ xT per span of <=4 token tiles. q^T (pre-scaled
     by 1/sqrt(dh)), k^T stay in SBUF; v tiles are PE-transposed into
     natural [k, d] layout, rows scaled by the key/pad mask, with the
     0/1 mask column appended (col 64/129).
  2. Attention in the TRANSPOSED orientation: S^T[k, q] = k^T.T @ q^T
     per 128-row k-tile (two heads run concurrently in the PE array via
     row tiling: head0 rows 0-63, head1 rows 64-127); exp(S^T - 10) on
     ACT straight out of PSUM, merged over k-tile pairs; causal via
     column trimming + an upper-triangular multiplicative mask on
     diagonal blocks.
  3. PV: out[65, q] = [v_h | mask01].T @ P^T accumulated over k-tiles.
     Row 64 is the softmax denominator. rq = 1 / (denom + pad_guard) is
     broadcast across partitions (GPSIMD) and multiplied in.
  4. out = y^T.T @ Wproj_rows -> partial [PT, 1024], host-summed.

Emission is software-pipelined: the next span's QKV matmuls and the
previous span's projection are interleaved into the current span's
attention pair-loop so the PE never drains while ACT works through the
exp stream (keeps HAM at 2.4 GHz). DMA queue plan: x stripes split
across the two fast HWDGE queues (sync low chunks / act high chunks,
x first in both), the q-weight tile first on act with the k/v weight
tiles slotted between the first two spans' stripes, small constants +
wproj on the GPSIMD SWDGE queue, outputs on sync (final spans on act). A burst
of dependency-free warm-up matmuls during the ~10us DMA preamble flips
the HAM clock gate to 2.4 GHz before real work starts. PSUM budget
(8 banks): mm 2 + st 2x2 + pv 2.
"""

import functools
import math

import numpy as np
import ml_dtypes

import concourse.bass as bass  # noqa: F401  (engine types via bacc)
import concourse.mybir as mybir
import concourse.tile as tile
from concourse import bacc
from concourse.bass_utils import run_bass_kernel_spmd
from concourse.masks import make_upper_triangular, make_identity

BF16 = mybir.dt.bfloat16
F32 = mybir.dt.float32
AF = mybir.ActivationFunctionType
OP = mybir.AluOpType

B, T, D, NH = 2, 2048, 1024, 16
DH = 64                  # head dim
HPC = 2                  # heads per core
NCORES = 8
P = 128
KC = D // P              # 8 contraction chunks for qkv
SPANT = 4                # q-tiles per attention span
QSCALE = 1.0 / math.sqrt(DH)
ESHIFT = -10.0           # constant exp shift; cancels in the softmax ratio
VW = 2 * (DH + 1)        # v_nat width: [v_h0 | mask | v_h1 | mask]


def _spans(nt):
    """Span table: (batch_tile_base, first_q_tile, n_q_tiles), longest
    batch first; plus each span's offset in the span-major x layout."""
    spans = []
    for b in sorted(range(len(nt)), key=lambda b: -nt[b]):
        tb = sum(nt[:b])
        for s0 in range(0, nt[b], SPANT):
            spans.append((tb, s0, min(SPANT, nt[b] - s0)))
    offs, o = [], 0
    for (tb, s0, nq) in spans:
        offs.append(o)
        o += KC * nq * P
    return spans, offs


def build(nt):
    """nt = (tiles_batch0, tiles_batch1) packed 128-token tiles."""
    nt = tuple(int(x) for x in nt)
    NTT = sum(nt)
    PT = NTT * P

    nc = bacc.Bacc(None)

    xT = nc.dram_tensor("xT", [P, KC * PT], BF16, kind="ExternalInput")
    wqkv = nc.dram_tensor("wqkv", [P, KC, 3 * P], BF16, kind="ExternalInput")
    bqkv = nc.dram_tensor("bqkv", [P, 3], F32, kind="ExternalInput")
    wproj = nc.dram_tensor("wproj", [P, D], BF16, kind="ExternalInput")
    mrowinv = nc.dram_tensor("mrowinv", [1, PT], F32, kind="ExternalInput")
    mcol = nc.dram_tensor("mcol", [P, NTT], F32, kind="ExternalInput")
    out = nc.dram_tensor("out", [PT, D], BF16, kind="ExternalOutput")

    spans, soffs = _spans(nt)

    with tile.TileContext(nc) as tc:
        with (
            tc.tile_pool(name="singles", bufs=1) as singles,
            tc.tile_pool(name="stage", bufs=2) as stage,
            tc.tile_pool(name="ptp", bufs=4) as ptp,
            tc.tile_pool(name="rows", bufs=2) as rows,
            tc.tile_pool(name="outs", bufs=3) as outs,
            tc.tile_pool(name="ps", bufs=2, space="PSUM") as ps,
        ):
            # ---- DMA queue plan: x split across the two fast HWDGE
            # queues (sync/act) with x FIRST in both; small constants up
            # front on act; bulk weights on the GPSIMD SWDGE queue;
            # outputs on sync behind the x stripes ----
            wqkv_sb = singles.tile([P, KC, 3 * P], BF16)
            nc.scalar.dma_start(out=wqkv_sb[:, :, 0:P], in_=wqkv[:, :, 0:P])
            bqkv_sb = singles.tile([P, 3], F32)
            nc.gpsimd.dma_start(out=bqkv_sb, in_=bqkv[:, :])
            mcol_sb = singles.tile([P, NTT], F32)
            nc.gpsimd.dma_start(out=mcol_sb, in_=mcol[:, :])
            mrowinv_sb = singles.tile([1, PT], F32)
            nc.gpsimd.dma_start(out=mrowinv_sb, in_=mrowinv[:, :])
            wproj_sb = singles.tile([P, D], BF16)
            nc.gpsimd.dma_start(out=wproj_sb, in_=wproj[:, :])
            # constants first: they are produced on GPSIMD/DVE and must
            # not queue behind the stripe-DMA issues below
            eshift_sb = singles.tile([P, 1], F32)
            nc.vector.memset(eshift_sb, ESHIFT)
            ut_sb = singles.tile([P, P], BF16)  # keep q >= k
            make_upper_triangular(nc, ut_sb, val=1.0, diag=True)
            ident = singles.tile([P, P], BF16)
            make_identity(nc, ident)
            # PE warm-up: dependency-free matmuls during the initial DMA
            # preamble flip HAM to 8/8 before real work starts
            warm_rhs = singles.tile([P, 512], BF16)
            nc.vector.memset(warm_rhs, 0.0)
            for w in range(20):
                wps = ps.tile([P, 512], F32, tag="st", bufs=2, name="warm")
                nc.tensor.matmul(wps[:], warm_rhs[:, 0:P], warm_rhs[:],
                                 start=True, stop=True)

            xT_sb = singles.tile([P, KC * PT], BF16)
            for i, (tb, s0, nq) in enumerate(spans):
                o, half = soffs[i], KC * nq * P // 2
                nc.sync.dma_start(out=xT_sb[:, o:o + half],
                                  in_=xT[:, o:o + half])
                nc.scalar.dma_start(out=xT_sb[:, o + half:o + 2 * half],
                                    in_=xT[:, o + half:o + 2 * half])
                if i < 2:  # k/v weights land between the early x stripes
                    m = i + 1
                    nc.scalar.dma_start(
                        out=wqkv_sb[:, :, m * P:(m + 1) * P],
                        in_=wqkv[:, :, m * P:(m + 1) * P])

            qT_sb = singles.tile([P, PT], BF16)  # rows: h0 d0..63 | h1 d0..63
            kT_sb = singles.tile([P, PT], BF16)
            yT_sb = singles.tile([P, PT], BF16)
            v_nat = singles.tile([P, NTT, VW], BF16)

            # ---- QKV production for one span, as resumable steps ----
            def qkv_steps(si):
                tb, s0, nq = spans[si]
                c0 = (tb + s0) * P
                W = nq * P
                xo = soffs[si]
                holder = []

                def emit_m(m):
                    pq = ps.tile([P, 512], F32, tag="mm", bufs=2, name="pq")
                    for k in range(KC):
                        nc.tensor.matmul(
                            pq[:, 0:W],
                            wqkv_sb[:, k, m * P:(m + 1) * P],
                            xT_sb[:, xo + k * W:xo + (k + 1) * W],
                            start=(k == 0), stop=(k == KC - 1),
                        )
                    if m == 0:
                        nc.scalar.activation(
                            qT_sb[:, c0:c0 + W], pq[:, 0:W], AF.Identity,
                            bias=bqkv_sb[:, 0:1], scale=QSCALE)
                    elif m == 1:
                        nc.scalar.activation(
                            kT_sb[:, c0:c0 + W], pq[:, 0:W], AF.Identity,
                            bias=bqkv_sb[:, 1:2])
                    else:
                        vst = stage.tile([P, 512], BF16, tag="vst", name="vst")
                        nc.scalar.activation(
                            vst[:, 0:W], pq[:, 0:W], AF.Identity,
                            bias=bqkv_sb[:, 2:3])
                        holder.append(vst)

                def emit_vtile(jj):
                    vst = holder[0]
                    jt = tb + s0 + jj
                    vtp = ps.tile([P, P], BF16, tag="mm", bufs=2, name="vtp")
                    nc.tensor.transpose(
                        vtp[:], vst[:, jj * P:(jj + 1) * P], ident[:])
                    nc.vector.tensor_scalar_mul(
                        v_nat[:, jt, 0:DH], vtp[:, 0:DH], mcol_sb[:, jt:jt + 1])
                    nc.vector.tensor_scalar_mul(
                        v_nat[:, jt, DH + 1:2 * DH + 1], vtp[:, DH:2 * DH],
                        mcol_sb[:, jt:jt + 1])
                    nc.gpsimd.tensor_copy(out=v_nat[:, jt, DH:DH + 1],
                                          in_=mcol_sb[:, jt:jt + 1])
                    nc.gpsimd.tensor_copy(out=v_nat[:, jt, VW - 1:VW],
                                          in_=mcol_sb[:, jt:jt + 1])

                steps = [lambda m=m: emit_m(m) for m in range(3)]
                steps += [lambda jj=jj: emit_vtile(jj) for jj in range(nq)]
                return steps

            # ---- projection of one span, as resumable steps ----
            def proj_steps(span_i):
                tb, s0, nq = spans[span_i]
                q0 = (tb + s0) * P
                late = span_i >= len(spans) - 2

                def emit_tt(tt):
                    trow = q0 + tt * P
                    ob = outs.tile([P, D], BF16, tag="ob", name="ob")
                    for half in range(2):
                        po = ps.tile([P, 512], F32, tag="mm", bufs=2, name="po")
                        nc.tensor.matmul(
                            po[:],
                            yT_sb[:, trow:trow + P],
                            wproj_sb[:, half * 512:(half + 1) * 512],
                            start=True, stop=True,
                        )
                        nc.vector.tensor_copy(
                            out=ob[:, half * 512:(half + 1) * 512], in_=po[:])
                    eng = nc.scalar if late else nc.sync
                    eng.dma_start(out=out[trow:trow + P, :], in_=ob)

                return [lambda tt=tt: emit_tt(tt) for tt in range(nq)]

            # ---- attention for one span, consuming filler steps ----
            def emit_attention(span_i, fillers):
                tb, s0, nq = spans[span_i]
                W = nq * P
                q0 = (tb + s0) * P
                nj = s0 + nq
                nf = len(fillers)
                pvs = [ps.tile([DH + 1, 512], F32, tag="pv", name=f"pv{h}")
                       for h in range(HPC)]
                for pi in range((nj + 1) // 2):
                    j0 = 2 * pi
                    djs = [dj for dj in range(2) if j0 + dj < nj]
                    offs = {dj: max(0, j0 + dj - s0) * P for dj in djs}
                    sts = []
                    for h in range(HPC):
                        st = ps.tile([P, 2 * W], F32, tag="st", bufs=2,
                                     name=f"st{h}")
                        hb = h * DH
                        for dj in djs:
                            off = offs[dj]
                            kb = (tb + j0 + dj) * P
                            nc.tensor.matmul(
                                st[:, dj * W + off:(dj + 1) * W],
                                kT_sb[hb:hb + DH, kb:kb + P],
                                qT_sb[hb:hb + DH, q0 + off:q0 + W],
                                start=True, stop=True,
                            )
                        sts.append(st)
                    for h in range(HPC):
                        pt = ptp.tile([P, 2 * W], BF16, tag="pt", name="pt")
                        if len(djs) == 2 and offs[1] == 0:
                            nc.scalar.activation(pt[:], sts[h][:], AF.Exp,
                                                 bias=eshift_sb[:])
                        else:
                            for dj in djs:
                                csl = slice(dj * W + offs[dj], (dj + 1) * W)
                                nc.scalar.activation(pt[:, csl],
                                                     sts[h][:, csl], AF.Exp,
                                                     bias=eshift_sb[:])
                        for dj in djs:
                            j = j0 + dj
                            off = offs[dj]
                            if j >= s0:  # diagonal block: keep q >= k
                                dsl = slice(dj * W + off, dj * W + off + P)
                                nc.vector.tensor_tensor(
                                    pt[:, dsl], pt[:, dsl], ut_sb[:], OP.mult)
                            vc0 = h * (DH + 1)
                            nc.tensor.matmul(
                                pvs[h][:, off:W],
                                v_nat[:, tb + j, vc0:vc0 + DH + 1],
                                pt[:, dj * W + off:(dj + 1) * W],
                                start=(j == 0), stop=(j == nj - 1),
                            )
                    # spread fillers across the pair loop, reserving one
                    # share to run during the normalization tail
                    npair = (nj + 1) // 2
                    take = (((pi + 1) * nf) // (npair + 1)
                            - (pi * nf) // (npair + 1))
                    for _ in range(take):
                        fillers.pop(0)()

                for h in range(HPC):
                    den = rows.tile([1, 512], F32, tag="den", name="den")
                    nc.vector.tensor_tensor(
                        den[:, 0:W], pvs[h][DH:DH + 1, 0:W],
                        mrowinv_sb[0:1, q0:q0 + W], OP.add)
                    rq = rows.tile([1, 512], F32, tag="rq", name="rq")
                    nc.vector.reciprocal_approx_fast(
                        out=rq[:, 0:W], in_=den[:, 0:W])
                    bc = rows.tile([DH, 512], F32, tag="bc", name="bc")
                    nc.gpsimd.partition_broadcast(bc[:, 0:W], rq[:, 0:W])
                    hb = h * DH
                    nc.vector.tensor_tensor(
                        yT_sb[hb:hb + DH, q0:q0 + W],
                        pvs[h][0:DH, 0:W], bc[:, 0:W], OP.mult)
                # leftover fillers give the PE work during the norm tail
                while fillers:
                    fillers.pop(0)()

            # ---- main pipeline ----
            for step in qkv_steps(0):
                step()
            pending = []
            for si in range(len(spans)):
                fillers = pending
                if si + 1 < len(spans):
                    fillers = fillers + qkv_steps(si + 1)
                emit_attention(si, fillers)
                pending = proj_steps(si)
            for step in pending:
                step()

    nc.finalize()
    return nc


@functools.lru_cache(maxsize=4)
def _built(nt):
    return build(nt)


def _plan(attention_mask):
    m = np.asarray(attention_mask) != 0
    idx = [np.nonzero(m[b])[0] for b in range(B)]
    nt = tuple(max(1, (len(i) + P - 1) // P) for i in idx)
    return idx, nt


def _prep_core(c, W_attn, b_attn, W_proj):
    bf = ml_dtypes.bfloat16
    q0 = c * HPC * DH
    qs = slice(q0, q0 + P)
    ks = slice(D + q0, D + q0 + P)
    vs = slice(2 * D + q0, 2 * D + q0 + P)
    wsl = np.concatenate(
        [W_attn[:, qs], W_attn[:, ks], W_attn[:, vs]], axis=1)  # [1024, 384]
    bq = b_attn[qs] * QSCALE
    return {
        "wqkv": np.ascontiguousarray(
            wsl.reshape(KC, P, 3 * P).transpose(1, 0, 2)).astype(bf),
        "bqkv": np.ascontiguousarray(
            np.stack([bq, b_attn[ks], b_attn[vs]], axis=1)).astype(np.float32),
        "wproj": np.ascontiguousarray(W_proj[qs, :]).astype(bf),
    }


def build_in_maps(x, attention_mask, W_attn, b_attn, W_proj):
    bf = ml_dtypes.bfloat16
    x = np.asarray(x, dtype=np.float32)
    W_attn = np.asarray(W_attn, dtype=np.float32)
    b_attn = np.asarray(b_attn, dtype=np.float32)
    W_proj = np.asarray(W_proj, dtype=np.float32)

    idx, nt = _plan(attention_mask)
    NTT = sum(nt)
    PT = NTT * P

    xp = np.zeros((PT, D), np.float32)
    qmask = np.zeros(PT, np.float32)
    base = 0
    for b in range(B):
        n = len(idx[b])
        xp[base:base + n] = x[b, idx[b]]
        qmask[base:base + n] = 1.0
        base += nt[b] * P

    spans, soffs = _spans(nt)
    blocks = []
    for (tb, s0, nq) in spans:
        r0 = (tb + s0) * P
        blk = xp[r0:r0 + nq * P].reshape(nq * P, KC, P).transpose(2, 1, 0)
        blocks.append(blk.reshape(P, KC * nq * P))
    xT_host = np.ascontiguousarray(np.concatenate(blocks, axis=1)).astype(bf)
    mrowinv = ((1.0 - qmask) * 1e30 + 1e-20).reshape(1, PT).astype(np.float32)
    mcol = np.ascontiguousarray(qmask.reshape(NTT, P).T).astype(np.float32)

    in_maps = []
    for c in range(NCORES):
        m = _prep_core(c, W_attn, b_attn, W_proj)
        m["xT"] = xT_host
        m["mrowinv"] = mrowinv
        m["mcol"] = mcol
        in_maps.append(m)
    return in_maps


def kernel(x, attention_mask, W_attn, b_attn, W_proj, b_proj):
    x = np.asarray(x, dtype=np.float32)
    attention_mask = np.asarray(attention_mask)
    b_proj = np.asarray(b_proj, dtype=np.float32)

    idx, nt = _plan(attention_mask)
    nc = _built(nt)
    in_maps = build_in_maps(x, attention_mask, W_attn, b_attn,
                            np.asarray(W_proj, dtype=np.float32))
    res = run_bass_kernel_spmd(nc, in_maps, core_ids=list(range(NCORES)))

    PT = sum(nt) * P
    acc = np.zeros((PT, D), dtype=np.float32)
    for c in range(NCORES):
        acc += res.results[c]["out"].astype(np.float32)

    y = np.empty((B, T, D), dtype=np.float32)
    y[:] = b_proj[None, None, :]
    base = 0
    for b in range(B):
        n = len(idx[b])
        y[b, idx[b]] = acc[base:base + n] + b_proj[None, :]
        base += nt[b] * P
    return y


# revision 45
# speedup vs baseline: 1.0323x; 1.0323x over previous
"""Trainium2 Bass kernel for CausalSelfAttention (B=2, T=2048, D=1024, H=16).

Sharding (8 cores): Megatron-style tensor parallel. Core c owns heads
{2c, 2c+1}: column-parallel c_attn (384 of 3072 output features),
full attention for its 2 heads x 2 batches, row-parallel c_proj
(128 of 1024 contraction rows). Host sums the 8 partial outputs and
adds b_proj.

Key optimization vs the dense version: the attention_mask zeroes ~50%
of tokens. Masked keys contribute nothing (key-masked), and masked
queries output exactly b_proj (their rows are zeroed before c_proj).
Since queries and keys share the same valid subset in order, packing
the valid tokens keeps the attention EXACTLY causal in packed space.
The kernel therefore runs on ~1024+pad tokens per batch instead of
2048: QKV/proj work halves, attention/softmax work quarters. The host
packs inputs / scatters outputs; the kernel is compile-specialized on
the per-batch packed tile counts (rebuilt if the mask pattern changes).

Device algorithm (per core), matmuls bf16, softmax f32:
  1. qkv^T = Wslice^T @ xT per span of <=4 token tiles. q^T (pre-scaled
     by 1/sqrt(dh)), k^T stay in SBUF; v tiles are PE-transposed into
     natural [k, d] layout, rows scaled by the key/pad mask, with the
     0/1 mask column appended (col 64/129).
  2. Attention in the TRANSPOSED orientation: S^T[k, q] = k^T.T @ q^T
     per 128-row k-tile (two heads run concurrently in the PE array via
     row tiling: head0 rows 0-63, head1 rows 64-127); exp(S^T - 10) on
     ACT straight out of PSUM, merged over k-tile pairs; causal via
     column trimming + an upper-triangular multiplicative mask on
     diagonal blocks.
  3. PV: out[65, q] = [v_h | mask01].T @ P^T accumulated over k-tiles.
     Row 64 is the softmax denominator. rq = 1 / (denom + pad_guard) is
     broadcast across partitions (GPSIMD) and multiplied in.
  4. out = y^T.T @ Wproj_rows -> partial [PT, 1024], host-summed.

Emission is software-pipelined: the next span's QKV matmuls and the
previous span's projection are interleaved into the current span's
attention pair-loop so the PE never drains while ACT works through the
exp stream (keeps HAM at 2.4 GHz). DMA queue plan: x stripes split
across the two fast HWDGE queues (sync low chunks / act high chunks,
x first in both), the q-weight tile first on act with the k/v weight
tiles slotted between the first two spans' stripes, small constants +
wproj on the GPSIMD SWDGE queue, outputs on sync (final spans on act). A burst
of dependency-free warm-up matmuls during the ~10us DMA preamble flips
the HAM clock gate to 2.4 GHz before real work starts. PSUM budget
(8 banks): mm 2 + st 2x2 + pv 2.
"""

import functools
import math

import numpy as np
import ml_dtypes

import concourse.bass as bass  # noqa: F401  (engine types via bacc)
import concourse.mybir as mybir
import concourse.tile as tile
from concourse import bacc
from concourse.bass_utils import run_bass_kernel_spmd
from concourse.masks import make_upper_triangular, make_identity

BF16 = mybir.dt.bfloat16
F32 = mybir.dt.float32
AF = mybir.ActivationFunctionType
OP = mybir.AluOpType

B, T, D, NH = 2, 2048, 1024, 16
DH = 64                  # head dim
HPC = 2                  # heads per core
NCORES = 8
P = 128
KC = D // P              # 8 contraction chunks for qkv
SPANT = 4                # q-tiles per attention span
QSCALE = 1.0 / math.sqrt(DH)
ESHIFT = -10.0           # constant exp shift; cancels in the softmax ratio
VW = 2 * (DH + 1)        # v_nat width: [v_h0 | mask | v_h1 | mask]


def build(nt):
    """nt = (tiles_batch0, tiles_batch1) packed 128-token tiles."""
    nt = tuple(int(x) for x in nt)
    NTT = sum(nt)
    PT = NTT * P

    nc = bacc.Bacc(None)

    xT = nc.dram_tensor("xT", [P, KC, PT], BF16, kind="ExternalInput")
    wqkv = nc.dram_tensor("wqkv", [P, KC, 3 * P], BF16, kind="ExternalInput")
    bqkv = nc.dram_tensor("bqkv", [P, 3], F32, kind="ExternalInput")
    wproj = nc.dram_tensor("wproj", [P, D], BF16, kind="ExternalInput")
    mrowinv = nc.dram_tensor("mrowinv", [1, PT], F32, kind="ExternalInput")
    mcol = nc.dram_tensor("mcol", [P, NTT], F32, kind="ExternalInput")
    out = nc.dram_tensor("out", [PT, D], BF16, kind="ExternalOutput")

    # span table: (batch_tile_base, first_q_tile_in_batch, n_q_tiles).
    # Batches ordered longest-first so the kernel's drain tail lands on a
    # full-width span (better overlap) instead of a 1-tile remainder.
    spans = []
    for b in sorted(range(len(nt)), key=lambda b: -nt[b]):
        tb = sum(nt[:b])
        for s0 in range(0, nt[b], SPANT):
            spans.append((tb, s0, min(SPANT, nt[b] - s0)))

    with tile.TileContext(nc) as tc:
        with (
            tc.tile_pool(name="singles", bufs=1) as singles,
            tc.tile_pool(name="stage", bufs=2) as stage,
            tc.tile_pool(name="ptp", bufs=4) as ptp,
            tc.tile_pool(name="rows", bufs=2) as rows,
            tc.tile_pool(name="outs", bufs=3) as outs,
            tc.tile_pool(name="ps", bufs=2, space="PSUM") as ps,
        ):
            # ---- DMA queue plan: x split across the two fast HWDGE
            # queues (sync/act) with x FIRST in both; small constants up
            # front on act; bulk weights on the GPSIMD SWDGE queue;
            # outputs on sync behind the x stripes ----
            wqkv_sb = singles.tile([P, KC, 3 * P], BF16)
            nc.scalar.dma_start(out=wqkv_sb[:, :, 0:P], in_=wqkv[:, :, 0:P])
            bqkv_sb = singles.tile([P, 3], F32)
            nc.gpsimd.dma_start(out=bqkv_sb, in_=bqkv[:, :])
            mcol_sb = singles.tile([P, NTT], F32)
            nc.gpsimd.dma_start(out=mcol_sb, in_=mcol[:, :])
            mrowinv_sb = singles.tile([1, PT], F32)
            nc.gpsimd.dma_start(out=mrowinv_sb, in_=mrowinv[:, :])
            wproj_sb = singles.tile([P, D], BF16)
            nc.gpsimd.dma_start(out=wproj_sb, in_=wproj[:, :])
            # constants first: they are produced on GPSIMD/DVE and must
            # not queue behind the stripe-DMA issues below
            eshift_sb = singles.tile([P, 1], F32)
            nc.vector.memset(eshift_sb, ESHIFT)
            ut_sb = singles.tile([P, P], BF16)  # keep q >= k
            make_upper_triangular(nc, ut_sb, val=1.0, diag=True)
            ident = singles.tile([P, P], BF16)
            make_identity(nc, ident)
            # PE warm-up: dependency-free matmuls during the initial DMA
            # preamble flip HAM to 8/8 before real work starts
            warm_rhs = singles.tile([P, 512], BF16)
            nc.vector.memset(warm_rhs, 0.0)
            for w in range(30):
                wps = ps.tile([P, 512], F32, tag="st", bufs=2, name="warm")
                nc.tensor.matmul(wps[:], warm_rhs[:, 0:P], warm_rhs[:],
                                 start=True, stop=True)

            xT_sb = singles.tile([P, KC, PT], BF16)
            for i, (tb, s0, nq) in enumerate(spans):
                csl = slice((tb + s0) * P, (tb + s0 + nq) * P)
                nc.sync.dma_start(out=xT_sb[:, 0:4, csl], in_=xT[:, 0:4, csl])
                nc.scalar.dma_start(out=xT_sb[:, 4:8, csl],
                                    in_=xT[:, 4:8, csl])
                if i < 2:  # k/v weights land between the early x stripes
                    m = i + 1
                    nc.scalar.dma_start(
                        out=wqkv_sb[:, :, m * P:(m + 1) * P],
                        in_=wqkv[:, :, m * P:(m + 1) * P])

            qT_sb = singles.tile([P, PT], BF16)  # rows: h0 d0..63 | h1 d0..63
            kT_sb = singles.tile([P, PT], BF16)
            yT_sb = singles.tile([P, PT], BF16)
            v_nat = singles.tile([P, NTT, VW], BF16)

            # ---- QKV production for one span, as resumable steps ----
            def qkv_steps(tb, s0, nq):
                c0 = (tb + s0) * P
                W = nq * P
                holder = []

                def emit_m(m):
                    pq = ps.tile([P, 512], F32, tag="mm", bufs=2, name="pq")
                    for k in range(KC):
                        nc.tensor.matmul(
                            pq[:, 0:W],
                            wqkv_sb[:, k, m * P:(m + 1) * P],
                            xT_sb[:, k, c0:c0 + W],
                            start=(k == 0), stop=(k == KC - 1),
                        )
                    if m == 0:
                        nc.scalar.activation(
                            qT_sb[:, c0:c0 + W], pq[:, 0:W], AF.Identity,
                            bias=bqkv_sb[:, 0:1], scale=QSCALE)
                    elif m == 1:
                        nc.scalar.activation(
                            kT_sb[:, c0:c0 + W], pq[:, 0:W], AF.Identity,
                            bias=bqkv_sb[:, 1:2])
                    else:
                        vst = stage.tile([P, 512], BF16, tag="vst", name="vst")
                        nc.scalar.activation(
                            vst[:, 0:W], pq[:, 0:W], AF.Identity,
                            bias=bqkv_sb[:, 2:3])
                        holder.append(vst)

                def emit_vtile(jj):
                    vst = holder[0]
                    jt = tb + s0 + jj
                    vtp = ps.tile([P, P], BF16, tag="mm", bufs=2, name="vtp")
                    nc.tensor.transpose(
                        vtp[:], vst[:, jj * P:(jj + 1) * P], ident[:])
                    nc.vector.tensor_scalar_mul(
                        v_nat[:, jt, 0:DH], vtp[:, 0:DH], mcol_sb[:, jt:jt + 1])
                    nc.vector.tensor_scalar_mul(
                        v_nat[:, jt, DH + 1:2 * DH + 1], vtp[:, DH:2 * DH],
                        mcol_sb[:, jt:jt + 1])
                    nc.gpsimd.tensor_copy(out=v_nat[:, jt, DH:DH + 1],
                                          in_=mcol_sb[:, jt:jt + 1])
                    nc.gpsimd.tensor_copy(out=v_nat[:, jt, VW - 1:VW],
                                          in_=mcol_sb[:, jt:jt + 1])

                steps = [lambda m=m: emit_m(m) for m in range(3)]
                steps += [lambda jj=jj: emit_vtile(jj) for jj in range(nq)]
                return steps

            # ---- projection of one span, as resumable steps ----
            def proj_steps(span_i):
                tb, s0, nq = spans[span_i]
                q0 = (tb + s0) * P
                late = span_i >= len(spans) - 2

                def emit_tt(tt):
                    trow = q0 + tt * P
                    ob = outs.tile([P, D], BF16, tag="ob", name="ob")
                    for half in range(2):
                        po = ps.tile([P, 512], F32, tag="mm", bufs=2, name="po")
                        nc.tensor.matmul(
                            po[:],
                            yT_sb[:, trow:trow + P],
                            wproj_sb[:, half * 512:(half + 1) * 512],
                            start=True, stop=True,
                        )
                        nc.vector.tensor_copy(
                            out=ob[:, half * 512:(half + 1) * 512], in_=po[:])
                    eng = nc.scalar if late else nc.sync
                    eng.dma_start(out=out[trow:trow + P, :], in_=ob)

                return [lambda tt=tt: emit_tt(tt) for tt in range(nq)]

            # ---- attention for one span, consuming filler steps ----
            def emit_attention(span_i, fillers):
                tb, s0, nq = spans[span_i]
                W = nq * P
                q0 = (tb + s0) * P
                nj = s0 + nq
                nf = len(fillers)
                pvs = [ps.tile([DH + 1, 512], F32, tag="pv", name=f"pv{h}")
                       for h in range(HPC)]
                for pi in range((nj + 1) // 2):
                    j0 = 2 * pi
                    djs = [dj for dj in range(2) if j0 + dj < nj]
                    offs = {dj: max(0, j0 + dj - s0) * P for dj in djs}
                    sts = []
                    for h in range(HPC):
                        st = ps.tile([P, 2 * W], F32, tag="st", bufs=2,
                                     name=f"st{h}")
                        hb = h * DH
                        for dj in djs:
                            off = offs[dj]
                            kb = (tb + j0 + dj) * P
                            nc.tensor.matmul(
                                st[:, dj * W + off:(dj + 1) * W],
                                kT_sb[hb:hb + DH, kb:kb + P],
                                qT_sb[hb:hb + DH, q0 + off:q0 + W],
                                start=True, stop=True,
                            )
                        sts.append(st)
                    for h in range(HPC):
                        pt = ptp.tile([P, 2 * W], BF16, tag="pt", name="pt")
                        if len(djs) == 2 and offs[1] == 0:
                            nc.scalar.activation(pt[:], sts[h][:], AF.Exp,
                                                 bias=eshift_sb[:])
                        else:
                            for dj in djs:
                                csl = slice(dj * W + offs[dj], (dj + 1) * W)
                                nc.scalar.activation(pt[:, csl],
                                                     sts[h][:, csl], AF.Exp,
                                                     bias=eshift_sb[:])
                        for dj in djs:
                            j = j0 + dj
                            off = offs[dj]
                            if j >= s0:  # diagonal block: keep q >= k
                                dsl = slice(dj * W + off, dj * W + off + P)
                                nc.vector.tensor_tensor(
                                    pt[:, dsl], pt[:, dsl], ut_sb[:], OP.mult)
                            vc0 = h * (DH + 1)
                            nc.tensor.matmul(
                                pvs[h][:, off:W],
                                v_nat[:, tb + j, vc0:vc0 + DH + 1],
                                pt[:, dj * W + off:(dj + 1) * W],
                                start=(j == 0), stop=(j == nj - 1),
                            )
                    # spread fillers across the pair loop, reserving one
                    # share to run during the normalization tail
                    npair = (nj + 1) // 2
                    take = (((pi + 1) * nf) // (npair + 1)
                            - (pi * nf) // (npair + 1))
                    for _ in range(take):
                        fillers.pop(0)()

                for h in range(HPC):
                    den = rows.tile([1, 512], F32, tag="den", name="den")
                    nc.vector.tensor_tensor(
                        den[:, 0:W], pvs[h][DH:DH + 1, 0:W],
                        mrowinv_sb[0:1, q0:q0 + W], OP.add)
                    rq = rows.tile([1, 512], F32, tag="rq", name="rq")
                    nc.vector.reciprocal_approx_fast(
                        out=rq[:, 0:W], in_=den[:, 0:W])
                    bc = rows.tile([DH, 512], F32, tag="bc", name="bc")
                    nc.gpsimd.partition_broadcast(bc[:, 0:W], rq[:, 0:W])
                    hb = h * DH
                    nc.vector.tensor_tensor(
                        yT_sb[hb:hb + DH, q0:q0 + W],
                        pvs[h][0:DH, 0:W], bc[:, 0:W], OP.mult)
                # leftover fillers give the PE work during the norm tail
                while fillers:
                    fillers.pop(0)()

            # ---- main pipeline ----
            for step in qkv_steps(*spans[0]):
                step()
            pending = []
            for si in range(len(spans)):
                fillers = pending
                if si + 1 < len(spans):
                    fillers = fillers + qkv_steps(*spans[si + 1])
                emit_attention(si, fillers)
                pending = proj_steps(si)
            for step in pending:
                step()

    nc.finalize()
    return nc


@functools.lru_cache(maxsize=4)
def _built(nt):
    return build(nt)


def _plan(attention_mask):
    m = np.asarray(attention_mask) != 0
    idx = [np.nonzero(m[b])[0] for b in range(B)]
    nt = tuple(max(1, (len(i) + P - 1) // P) for i in idx)
    return idx, nt


def _prep_core(c, W_attn, b_attn, W_proj):
    bf = ml_dtypes.bfloat16
    q0 = c * HPC * DH
    qs = slice(q0, q0 + P)
    ks = slice(D + q0, D + q0 + P)
    vs = slice(2 * D + q0, 2 * D + q0 + P)
    wsl = np.concatenate(
        [W_attn[:, qs], W_attn[:, ks], W_attn[:, vs]], axis=1)  # [1024, 384]
    bq = b_attn[qs] * QSCALE
    return {
        "wqkv": np.ascontiguousarray(
            wsl.reshape(KC, P, 3 * P).transpose(1, 0, 2)).astype(bf),
        "bqkv": np.ascontiguousarray(
            np.stack([bq, b_attn[ks], b_attn[vs]], axis=1)).astype(np.float32),
        "wproj": np.ascontiguousarray(W_proj[qs, :]).astype(bf),
    }


def build_in_maps(x, attention_mask, W_attn, b_attn, W_proj):
    bf = ml_dtypes.bfloat16
    x = np.asarray(x, dtype=np.float32)
    W_attn = np.asarray(W_attn, dtype=np.float32)
    b_attn = np.asarray(b_attn, dtype=np.float32)
    W_proj = np.asarray(W_proj, dtype=np.float32)

    idx, nt = _plan(attention_mask)
    NTT = sum(nt)
    PT = NTT * P

    xp = np.zeros((PT, D), np.float32)
    qmask = np.zeros(PT, np.float32)
    base = 0
    for b in range(B):
        n = len(idx[b])
        xp[base:base + n] = x[b, idx[b]]
        qmask[base:base + n] = 1.0
        base += nt[b] * P

    xT_host = np.ascontiguousarray(
        xp.reshape(PT, KC, P).transpose(2, 1, 0)).astype(bf)
    mrowinv = ((1.0 - qmask) * 1e30 + 1e-20).reshape(1, PT).astype(np.float32)
    mcol = np.ascontiguousarray(qmask.reshape(NTT, P).T).astype(np.float32)

    in_maps = []
    for c in range(NCORES):
        m = _prep_core(c, W_attn, b_attn, W_proj)
        m["xT"] = xT_host
        m["mrowinv"] = mrowinv
        m["mcol"] = mcol
        in_maps.append(m)
    return in_maps


def kernel(x, attention_mask, W_attn, b_attn, W_proj, b_proj):
    x = np.asarray(x, dtype=np.float32)
    attention_mask = np.asarray(attention_mask)
    b_proj = np.asarray(b_proj, dtype=np.float32)

    idx, nt = _plan(attention_mask)
    nc = _built(nt)
    in_maps = build_in_maps(x, attention_mask, W_attn, b_attn,
                            np.asarray(W_proj, dtype=np.float32))
    res = run_bass_kernel_spmd(nc, in_maps, core_ids=list(range(NCORES)))

    PT = sum(nt) * P
    acc = np.zeros((PT, D), dtype=np.float32)
    for c in range(NCORES):
        acc += res.results[c]["out"].astype(np.float32)

    y = np.empty((B, T, D), dtype=np.float32)
    y[:] = b_proj[None, None, :]
    base = 0
    for b in range(B):
        n = len(idx[b])
        y[b, idx[b]] = acc[base:base + n] + b_proj[None, :]
        base += nt[b] * P
    return y
